# revision 1
# baseline (speedup 1.0000x reference)
"""Bezier curve Gaussian rasterization on 8 Trainium2 NeuronCores.

Problem: curves [8,4,2] -> raster [512,512] where
    out[b,a] = sum_s Ey[b,s] * Ex[a,s]
    Ex[a,s] = exp(-5000*(x_s - a/512)^2),  x_s = cubic Bezier samples,
    T = 8 curves x 128 t-samples = 1024.

Strategy (no collectives -- their ~10us floor dwarfs this kernel):
shard OUTPUT ROWS b across the 8 cores. Core k computes
out[64k:64k+64, :] with the s-contraction (1024) done as 8 accumulating
float32r PE matmuls. Each core computes the full ExT (s on partitions,
8 tiles of [128, 512]) plus its own 64-wide Ey slice:
  d^2 via a custom DVE op select(1, sq(Idx - s0), in0) -- the pixel grid
  comes from the DVE's index scan (no grid input tensor); a few y-parts
  run on ACT as Square(iota + bias) for engine balance; exp on ACT;
  Bezier sampling via a tiny PE matmul against a baked Bernstein basis
  (the only input DMA, hoisted before the framework entry barrier).

kernel(curves) -> np.ndarray [512,512] float32.
"""
import sys
import types

import numpy as np

RES = 512
STEPS = 128
N_CURVES = 8
N_CORES = 8
BROWS = RES // N_CORES  # 64 output rows per core
W = RES + BROWS  # 576 = per-tile width (x part | y part)
SIGMA = 0.01
# exp scale in pixel units: -(1/(2 sigma^2)) / RES^2
EXP_SCALE = -1.0 / (2.0 * SIGMA * SIGMA) / (RES * RES)

_CACHE = {}
N_ACT_Y = 4  # tiles whose y-square runs on ACT instead of DVE
N_WARM = 5  # PE warm-up dummy matmuls


def _install_ntff_hook():
    """Provide antenv.axon_hooks (missing in this image) so NTFF
    profiling via run_bass_kernel_spmd(trace=True) works."""
    try:
        import antenv
    except ImportError:
        return
    if "antenv.axon_hooks" in sys.modules:
        return
    mod = types.ModuleType("antenv.axon_hooks")
    _state = {"hook": None}
    mod.set_axon_ntff_profile_hook = lambda h: _state.__setitem__("hook", h)
    mod.get_axon_ntff_profile_hook = lambda: _state["hook"]
    sys.modules["antenv.axon_hooks"] = mod
    antenv.axon_hooks = mod
    try:
        from trn_agent_boot.trn_boot import _ntff_profile_via_ctypes

        hook = _ntff_profile_via_ctypes("/opt/axon/libaxon_pjrt.so")
        if hook is not None:
            mod.set_axon_ntff_profile_hook(hook)
    except Exception:
        pass


def _get_sqidx():
    """Register (once) a custom DVE op: out[p, k] = (k - s0[p])^2.

    The element index k comes from the DVE scan unit (Idx); in0 is only
    consumed to drive the stream (its value is muxed away by the select),
    so the op needs no real grid input. One Vector instruction replaces
    iota + subtract + square.
    """
    if "sqidx" in _CACHE:
        return _CACHE["sqidx"]
    from concourse import dve_ops
    from concourse.dve_spec import (
        Spec, Src0, C0, Idx, One, sq, select, lower, _has_src1,
    )
    from concourse.dve_uop import DveOpSpec

    name = "SQIDX_ANT"

    def ref(in0, in1, s0, s1, imm2):
        idx = np.arange(in0.shape[-1], dtype=np.float32)
        return (idx[None, :] - s0) ** 2

    spec = Spec(body=select(One, sq(Idx - C0), Src0), reference=ref)
    row = dve_ops._CUSTOM_DVE_ROW_BASE + len(dve_ops.OPS)
    assert row < 0x20
    dve_ops._SUB_OPCODE_FOR_NAME[name] = row
    shas = {}
    for ver in ("v3", "v4"):
        try:
            s = DveOpSpec(name=name, opcode=row, uops=lower(spec, ver=ver),
                          rd1_en=_has_src1(spec))
            shas[ver] = s.sha(ver)
        except Exception:
            pass
    op = dve_ops.DveOp(name, spec, subdim=False, uops_sha=shas)
    dve_ops.OPS.append(op)
    dve_ops.CUSTOM_DVE_SPECS[name] = spec
    _CACHE["sqidx"] = op
    return op


def _bernstein_basis() -> np.ndarray:
    """bt [4, 128]: bt[j, p] = B_j(t_p), t = linspace(0,1,128) fp32."""
    t = np.linspace(0.0, 1.0, STEPS, dtype=np.float32).astype(np.float64)
    u = 1.0 - t
    bt = np.stack([u**3, 3 * t * u**2, 3 * t**2 * u, t**3])
    return bt.astype(np.float32)


def build_bass():
    import concourse.bass as bass
    import concourse.tile as tile
    from concourse import bacc, mybir

    sqidx = _get_sqidx()

    nc = bacc.Bacc("TRN2", target_bir_lowering=False, debug=False, num_devices=N_CORES)
    # input layout [4, 25+128]: cols 0..7: 512*x_j ctrl pts; col 8:
    # 512*x_7-256 (tile-7 right-half base); cols 9..16: 512*y_j-64k;
    # cols 17..24: -(512*y_j-64k); cols 25..152: Bernstein basis bt [4,128]
    NCV = 3 * N_CURVES + 1
    NX = N_CURVES + 1  # x block width
    XCOL7R = N_CURVES
    cvbt = nc.dram_tensor("cvbt", [4, NCV + STEPS], mybir.dt.float32, kind="ExternalInput").ap()
    out = nc.dram_tensor("out", [BROWS, RES], mybir.dt.float32, kind="ExternalOutput").ap()

    f32 = mybir.dt.float32
    f32r = mybir.dt.float32r
    Exp = mybir.ActivationFunctionType.Exp
    Square = mybir.ActivationFunctionType.Square

    cvbt_sb_t = nc.alloc_sbuf_tensor("cvbt_sb_raw", [4, NCV + STEPS], f32)
    cvbt_sem = nc.alloc_semaphore("cvbt_in_sem")
    cvbt_sb = cvbt_sb_t.ap()
    cv_dma = nc.sync.dma_start(out=cvbt_sb[:], in_=cvbt[:]).then_inc(cvbt_sem, 16)

    deferred_waits = []

    def guard(engine, sem):
        deferred_waits.append((engine.wait_ge(sem, 0), sem))

    with tile.TileContext(nc) as tc:
        with (
            tc.tile_pool(name="const", bufs=1) as cpool,
            tc.tile_pool(name="d", bufs=3) as dpool,
            tc.tile_pool(name="e", bufs=8) as epool,
            tc.tile_pool(name="res", bufs=1) as rpool,
            tc.tile_pool(name="psum", bufs=1, space="PSUM") as ppool,
            tc.tile_pool(name="warmp", bufs=1, space="PSUM") as wpool,
            tc.tile_pool(name="psum_out", bufs=1, space="PSUM") as opool,
        ):
            # Dummy first ACT op with no DMA dependency: anchors the ~1.3us
            # ACT_TABLE_LOAD at body start instead of behind a wait.
            warm = cpool.tile([1, 2], f32)
            nc.vector.memset(warm[:], 0.0)
            nc.scalar.activation(warm[:, 1:2], warm[:, 0:1], Exp)

            # pixel row index 0..63 for the ACT y-path
            iay = cpool.tile([STEPS, BROWS], f32)
            nc.gpsimd.iota(iay[:], [[1, BROWS]], channel_multiplier=0,
                           allow_small_or_imprecise_dtypes=True)

            # Bezier sampling matmul -> psum_xy [128, 25]
            psum_xy = ppool.tile([STEPS, NCV], f32)
            guard(nc.tensor, cvbt_sem)
            nc.tensor.matmul(
                psum_xy[:], lhsT=cvbt_sb[:, NCV:], rhs=cvbt_sb[:, 0:NCV],
                start=True, stop=True,
            )
            xy_sb = cpool.tile([STEPS, NCV], f32)
            nc.vector.tensor_copy(out=xy_sb[:], in_=psum_xy[:])

            # PE warm-up: garbage matmuls into a scratch bank keep the PE
            # busy so the HAM clock-gate opens before the real matmuls.
            garb = cpool.tile([STEPS, RES], f32)
            nc.vector.memset(garb[:], 0.0)
            psum_warm = wpool.tile([STEPS, RES], f32)
            for _ in range(N_WARM):
                nc.tensor.matmul(
                    psum_warm[:],
                    lhsT=garb[:, 0:STEPS].bitcast(f32r),
                    rhs=garb[:].bitcast(f32r),
                    start=True, stop=True, skip_group_check=True,
                )

            # Two PSUM banks (left/right raster halves): the final copy of one
            # half can overlap the other half's last matmuls without the
            # PE-write/engine-read same-bank serialization.
            H = RES // 2
            psum_l = opool.tile([BROWS, H], f32, tag="outL")
            psum_r = opool.tile([BROWS, H], f32, tag="outR")

            for j in range(N_CURVES - 1):
                d = dpool.tile([STEPS, W], f32)
                # y part: d[:, 512:576] = (r - (512*y_j - 64k))^2
                if j < N_ACT_Y:
                    nc.scalar.activation(
                        d[:, RES:W], iay[:], Square,
                        bias=xy_sb[:, 17 + j : 18 + j], scale=1.0,
                    )
                else:
                    nc.vector._custom_dve(
                        sqidx,
                        out=d[:, RES:W],
                        in0=d[:, RES:W],
                        s0=xy_sb[:, 9 + j : 10 + j],
                    )
                # x part: d[:, 0:512] = (a - 512*x_j)^2
                nc.vector._custom_dve(
                    sqidx,
                    out=d[:, 0:RES],
                    in0=d[:, 0:RES],
                    s0=xy_sb[:, j : j + 1],
                )
                e = epool.tile([STEPS, W], f32r)
                nc.scalar.activation(e[:], d[:], Exp, scale=EXP_SCALE)
                nc.tensor.matmul(
                    psum_l[:], lhsT=e[:, RES:W], rhs=e[:, 0:H],
                    start=(j == 0), stop=False,
                )
                nc.tensor.matmul(
                    psum_r[:], lhsT=e[:, RES:W], rhs=e[:, H:RES],
                    start=(j == 0), stop=False,
                )

            # Tile 7 drives the kernel tail: lay it out [y | x-left | x-right]
            # and split its x into two half-width ops (the extra input column
            # carries 512*x_7 - 256 so the right half's index base is zero),
            # so each half's exp -> matmul -> copy -> store chain starts as
            # soon as its half of the distance field exists.
            j = N_CURVES - 1
            d = dpool.tile([STEPS, W], f32)
            nc.vector._custom_dve(  # y: d[:, 0:64]
                sqidx, out=d[:, 0:BROWS], in0=d[:, 0:BROWS],
                s0=xy_sb[:, 9 + j : 10 + j],
            )
            nc.vector._custom_dve(  # x-left: d[:, 64:320] (a = 0..255)
                sqidx, out=d[:, BROWS : BROWS + H], in0=d[:, BROWS : BROWS + H],
                s0=xy_sb[:, j : j + 1],
            )
            nc.vector._custom_dve(  # x-right: d[:, 320:576] (a = 256..511)
                sqidx, out=d[:, BROWS + H : W], in0=d[:, BROWS + H : W],
                s0=xy_sb[:, XCOL7R : XCOL7R + 1],
            )
            e = epool.tile([STEPS, W], f32r)
            res_sb = rpool.tile([BROWS, RES], f32)
            nc.scalar.activation(e[:, 0 : BROWS + H], d[:, 0 : BROWS + H], Exp, scale=EXP_SCALE)
            nc.tensor.matmul(
                psum_l[:], lhsT=e[:, 0:BROWS], rhs=e[:, BROWS : BROWS + H],
                start=False, stop=True,
            )
            nc.scalar.copy(out=res_sb[:, 0:H], in_=psum_l[:])
            nc.sync.dma_start(out=out[:, 0:H], in_=res_sb[:, 0:H])
            nc.scalar.activation(e[:, BROWS + H : W], d[:, BROWS + H : W], Exp, scale=EXP_SCALE)
            nc.tensor.matmul(
                psum_r[:], lhsT=e[:, 0:BROWS], rhs=e[:, BROWS + H : W],
                start=False, stop=True,
            )
            nc.vector.tensor_copy(out=res_sb[:, H:RES], in_=psum_r[:])
            nc.scalar.dma_start(out=out[:, H:RES], in_=res_sb[:, H:RES])

    for inst, sem in deferred_waits:
        for wt in inst.ins.sync_info.on_wait:
            if wt.id == sem.num:
                wt.wait_value = 16

    # Hoist the cvbt DMA to the top of the main block, before the framework
    # entry barrier, so it overlaps the per-engine NRT preamble.
    main_blk = nc.m.functions[0].blocks[0]
    insts = main_blk.instructions
    idx = next(i for i, ins in enumerate(insts) if ins.name == cv_dma.ins.name)
    dma_ins = insts.pop(idx)
    insts.insert(1, dma_ins)  # right after the Call
    main_blk.instructions = insts

    # After the tile exit barriers: reset the manual input sem so a
    # re-execution of this loaded NEFF sees it at zero.
    nc.sync.sem_clear(cvbt_sem)

    nc.compile()
    return nc


def _make_inputs(curves: np.ndarray):
    """Per-core input maps."""
    bt = _bernstein_basis()
    xs = (RES * curves[:, :, 0]).astype(np.float32)  # [8,4] = 512*x control pts
    ys = (RES * curves[:, :, 1]).astype(np.float32)

    in_maps = []
    for k in range(N_CORES):
        ysk = ys.T - np.float32(BROWS * k)
        cvbt = np.empty((4, 3 * N_CURVES + 1 + STEPS), dtype=np.float32)
        cvbt[:, 0:N_CURVES] = xs.T
        cvbt[:, N_CURVES] = xs.T[:, N_CURVES - 1] - np.float32(RES // 2)
        cvbt[:, N_CURVES + 1 : 2 * N_CURVES + 1] = ysk
        cvbt[:, 2 * N_CURVES + 1 : 3 * N_CURVES + 1] = -ysk
        cvbt[:, 3 * N_CURVES + 1 :] = bt
        in_maps.append({"cvbt": cvbt})
    return in_maps


def kernel(curves: np.ndarray, trace: bool = False, tmpdir: str | None = None):
    _install_ntff_hook()
    from concourse.bass_utils import run_bass_kernel_spmd

    if "nc" not in _CACHE:
        _CACHE["nc"] = build_bass()
    nc = _CACHE["nc"]

    in_maps = _make_inputs(np.asarray(curves, dtype=np.float32))
    kw = {}
    if trace:
        import concourse.bass_utils as bu

        bu.upload_artifacts = lambda d: d  # no bucket in this container
        kw = {"trace": True, "tmpdir": tmpdir}
    res = run_bass_kernel_spmd(nc, in_maps, core_ids=list(range(N_CORES)), **kw)

    full = np.concatenate([res.results[k]["out"] for k in range(N_CORES)], axis=0)
    if trace:
        return full, res
    return full



# revision 3
# speedup vs baseline: 1.1638x; 1.1638x over previous
"""Bezier curve Gaussian rasterization on 8 Trainium2 NeuronCores.

Problem: curves [8,4,2] -> raster [512,512] where
    out[b,a] = sum_s Ey[b,s] * Ex[a,s]
    Ex[a,s] = exp(-5000*(x_s - a/512)^2),  x_s = cubic Bezier samples,
    T = 8 curves x 128 t-samples = 1024.

Strategy (no collectives -- their ~10us floor dwarfs this kernel):
shard OUTPUT ROWS b across the 8 cores. Core k computes
out[64k:64k+64, :] with the s-contraction (1024) done as 8 accumulating
float32r PE matmuls. Each core computes the full ExT (s on partitions,
8 tiles of [128, 512]) plus its own 64-wide Ey slice:
  d^2 via a custom DVE op select(1, sq(Idx - s0), in0) -- the pixel grid
  comes from the DVE's index scan (no grid input tensor); a few y-parts
  run on ACT as Square(iota + bias) for engine balance; exp on ACT;
  Bezier sampling via a tiny PE matmul against a baked Bernstein basis
  (the only input DMA, hoisted before the framework entry barrier).

kernel(curves) -> np.ndarray [512,512] float32.
"""
import sys
import types

import numpy as np

RES = 512
STEPS = 128
N_CURVES = 8
N_CORES = 8
BROWS = RES // N_CORES  # 64 output rows per core
W = RES + BROWS  # 576 = per-tile width (x part | y part)
SIGMA = 0.01
# exp scale in pixel units: -(1/(2 sigma^2)) / RES^2
EXP_SCALE = -1.0 / (2.0 * SIGMA * SIGMA) / (RES * RES)

_CACHE = {}
N_ACT_Y = 4  # tiles whose y-square runs on ACT instead of DVE
N_WARM = 5  # PE warm-up dummy matmuls


def _install_walrus_args_patch():
    """Append walrus flags that shrink the NEFF's fixed preamble/postamble.

    The stock postamble zeroes every semaphore 2..255 as individual
    EVENT_SEMAPHORE writes split across the 5 engines (~8us of teardown
    that counts toward measured exec time). Capping --max-sem-num shrinks
    that sweep; our own kernel sems are cleared explicitly.
    """
    if _CACHE.get("walrus_patched"):
        return
    import concourse.bass_utils as bu

    orig = bu.get_walrus_args

    def patched(*a, **kw):
        return [*orig(*a, **kw), "--max-sem-num=16"]

    bu.get_walrus_args = patched
    _CACHE["walrus_patched"] = True


def _install_ntff_hook():
    """Provide antenv.axon_hooks (missing in this image) so NTFF
    profiling via run_bass_kernel_spmd(trace=True) works."""
    try:
        import antenv
    except ImportError:
        return
    if "antenv.axon_hooks" in sys.modules:
        return
    mod = types.ModuleType("antenv.axon_hooks")
    _state = {"hook": None}
    mod.set_axon_ntff_profile_hook = lambda h: _state.__setitem__("hook", h)
    mod.get_axon_ntff_profile_hook = lambda: _state["hook"]
    sys.modules["antenv.axon_hooks"] = mod
    antenv.axon_hooks = mod
    try:
        from trn_agent_boot.trn_boot import _ntff_profile_via_ctypes

        hook = _ntff_profile_via_ctypes("/opt/axon/libaxon_pjrt.so")
        if hook is not None:
            mod.set_axon_ntff_profile_hook(hook)
    except Exception:
        pass


def _get_sqidx():
    """Register (once) a custom DVE op: out[p, k] = (k - s0[p])^2.

    The element index k comes from the DVE scan unit (Idx); in0 is only
    consumed to drive the stream (its value is muxed away by the select),
    so the op needs no real grid input. One Vector instruction replaces
    iota + subtract + square.
    """
    if "sqidx" in _CACHE:
        return _CACHE["sqidx"]
    from concourse import dve_ops
    from concourse.dve_spec import (
        Spec, Src0, C0, Idx, One, sq, select, lower, _has_src1,
    )
    from concourse.dve_uop import DveOpSpec

    name = "SQIDX_ANT"

    def ref(in0, in1, s0, s1, imm2):
        idx = np.arange(in0.shape[-1], dtype=np.float32)
        return (idx[None, :] - s0) ** 2

    spec = Spec(body=select(One, sq(Idx - C0), Src0), reference=ref)
    row = dve_ops._CUSTOM_DVE_ROW_BASE + len(dve_ops.OPS)
    assert row < 0x20
    dve_ops._SUB_OPCODE_FOR_NAME[name] = row
    shas = {}
    for ver in ("v3", "v4"):
        try:
            s = DveOpSpec(name=name, opcode=row, uops=lower(spec, ver=ver),
                          rd1_en=_has_src1(spec))
            shas[ver] = s.sha(ver)
        except Exception:
            pass
    op = dve_ops.DveOp(name, spec, subdim=False, uops_sha=shas)
    dve_ops.OPS.append(op)
    dve_ops.CUSTOM_DVE_SPECS[name] = spec
    _CACHE["sqidx"] = op
    return op


def _bernstein_basis() -> np.ndarray:
    """bt [4, 128]: bt[j, p] = B_j(t_p), t = linspace(0,1,128) fp32."""
    t = np.linspace(0.0, 1.0, STEPS, dtype=np.float32).astype(np.float64)
    u = 1.0 - t
    bt = np.stack([u**3, 3 * t * u**2, 3 * t**2 * u, t**3])
    return bt.astype(np.float32)


def build_bass():
    import concourse.bass as bass
    import concourse.tile as tile
    from concourse import bacc, mybir

    sqidx = _get_sqidx()

    nc = bacc.Bacc("TRN2", target_bir_lowering=False, debug=False, num_devices=N_CORES)
    # input layout [4, 25+128]: cols 0..7: 512*x_j ctrl pts; col 8:
    # 512*x_7-256 (tile-7 right-half base); cols 9..16: 512*y_j-64k;
    # cols 17..24: -(512*y_j-64k); cols 25..152: Bernstein basis bt [4,128]
    NCV = 3 * N_CURVES + 1
    NX = N_CURVES + 1  # x block width
    XCOL7R = N_CURVES
    cvbt = nc.dram_tensor("cvbt", [4, NCV + STEPS], mybir.dt.float32, kind="ExternalInput").ap()
    out = nc.dram_tensor("out", [BROWS, RES], mybir.dt.float32, kind="ExternalOutput").ap()

    f32 = mybir.dt.float32
    f32r = mybir.dt.float32r
    Exp = mybir.ActivationFunctionType.Exp
    Square = mybir.ActivationFunctionType.Square

    cvbt_sb_t = nc.alloc_sbuf_tensor("cvbt_sb_raw", [4, NCV + STEPS], f32)
    cvbt_sem = nc.alloc_semaphore("cvbt_in_sem")
    cvbt_sb = cvbt_sb_t.ap()
    cv_dma = nc.sync.dma_start(out=cvbt_sb[:], in_=cvbt[:]).then_inc(cvbt_sem, 16)

    deferred_waits = []

    def guard(engine, sem):
        deferred_waits.append((engine.wait_ge(sem, 0), sem))

    with tile.TileContext(nc) as tc:
        with (
            tc.tile_pool(name="const", bufs=1) as cpool,
            tc.tile_pool(name="d", bufs=3) as dpool,
            tc.tile_pool(name="e", bufs=8) as epool,
            tc.tile_pool(name="res", bufs=1) as rpool,
            tc.tile_pool(name="psum", bufs=1, space="PSUM") as ppool,
            tc.tile_pool(name="warmp", bufs=1, space="PSUM") as wpool,
            tc.tile_pool(name="psum_out", bufs=1, space="PSUM") as opool,
        ):
            # Dummy first ACT op with no DMA dependency: anchors the ~1.3us
            # ACT_TABLE_LOAD at body start instead of behind a wait.
            warm = cpool.tile([1, 2], f32)
            nc.vector.memset(warm[:], 0.0)
            nc.scalar.activation(warm[:, 1:2], warm[:, 0:1], Exp)

            # pixel row index 0..63 for the ACT y-path
            iay = cpool.tile([STEPS, BROWS], f32)
            nc.gpsimd.iota(iay[:], [[1, BROWS]], channel_multiplier=0,
                           allow_small_or_imprecise_dtypes=True)

            # Bezier sampling matmul -> psum_xy [128, 25]
            psum_xy = ppool.tile([STEPS, NCV], f32)
            guard(nc.tensor, cvbt_sem)
            nc.tensor.matmul(
                psum_xy[:], lhsT=cvbt_sb[:, NCV:], rhs=cvbt_sb[:, 0:NCV],
                start=True, stop=True,
            )
            xy_sb = cpool.tile([STEPS, NCV], f32)
            nc.vector.tensor_copy(out=xy_sb[:], in_=psum_xy[:])

            # PE warm-up: garbage matmuls into a scratch bank keep the PE
            # busy so the HAM clock-gate opens before the real matmuls.
            garb = cpool.tile([STEPS, RES], f32)
            nc.vector.memset(garb[:], 0.0)
            psum_warm = wpool.tile([STEPS, RES], f32)
            for _ in range(N_WARM):
                nc.tensor.matmul(
                    psum_warm[:],
                    lhsT=garb[:, 0:STEPS].bitcast(f32r),
                    rhs=garb[:].bitcast(f32r),
                    start=True, stop=True, skip_group_check=True,
                )

            # Two PSUM banks (left/right raster halves): the final copy of one
            # half can overlap the other half's last matmuls without the
            # PE-write/engine-read same-bank serialization.
            H = RES // 2
            psum_l = opool.tile([BROWS, H], f32, tag="outL")
            psum_r = opool.tile([BROWS, H], f32, tag="outR")

            for j in range(N_CURVES - 1):
                d = dpool.tile([STEPS, W], f32)
                # y part: d[:, 512:576] = (r - (512*y_j - 64k))^2
                if j < N_ACT_Y:
                    nc.scalar.activation(
                        d[:, RES:W], iay[:], Square,
                        bias=xy_sb[:, 17 + j : 18 + j], scale=1.0,
                    )
                else:
                    nc.vector._custom_dve(
                        sqidx,
                        out=d[:, RES:W],
                        in0=d[:, RES:W],
                        s0=xy_sb[:, 9 + j : 10 + j],
                    )
                # x part: d[:, 0:512] = (a - 512*x_j)^2
                nc.vector._custom_dve(
                    sqidx,
                    out=d[:, 0:RES],
                    in0=d[:, 0:RES],
                    s0=xy_sb[:, j : j + 1],
                )
                e = epool.tile([STEPS, W], f32r)
                nc.scalar.activation(e[:], d[:], Exp, scale=EXP_SCALE)
                nc.tensor.matmul(
                    psum_l[:], lhsT=e[:, RES:W], rhs=e[:, 0:H],
                    start=(j == 0), stop=False,
                )
                nc.tensor.matmul(
                    psum_r[:], lhsT=e[:, RES:W], rhs=e[:, H:RES],
                    start=(j == 0), stop=False,
                )

            # Tile 7 drives the kernel tail: lay it out [y | x-left | x-right]
            # and split its x into two half-width ops (the extra input column
            # carries 512*x_7 - 256 so the right half's index base is zero),
            # so each half's exp -> matmul -> copy -> store chain starts as
            # soon as its half of the distance field exists.
            j = N_CURVES - 1
            d = dpool.tile([STEPS, W], f32)
            nc.vector._custom_dve(  # y: d[:, 0:64]
                sqidx, out=d[:, 0:BROWS], in0=d[:, 0:BROWS],
                s0=xy_sb[:, 9 + j : 10 + j],
            )
            nc.vector._custom_dve(  # x-left: d[:, 64:320] (a = 0..255)
                sqidx, out=d[:, BROWS : BROWS + H], in0=d[:, BROWS : BROWS + H],
                s0=xy_sb[:, j : j + 1],
            )
            nc.vector._custom_dve(  # x-right: d[:, 320:576] (a = 256..511)
                sqidx, out=d[:, BROWS + H : W], in0=d[:, BROWS + H : W],
                s0=xy_sb[:, XCOL7R : XCOL7R + 1],
            )
            e = epool.tile([STEPS, W], f32r)
            res_sb = rpool.tile([BROWS, RES], f32)
            nc.scalar.activation(e[:, 0 : BROWS + H], d[:, 0 : BROWS + H], Exp, scale=EXP_SCALE)
            nc.tensor.matmul(
                psum_l[:], lhsT=e[:, 0:BROWS], rhs=e[:, BROWS : BROWS + H],
                start=False, stop=True,
            )
            nc.scalar.copy(out=res_sb[:, 0:H], in_=psum_l[:])
            nc.sync.dma_start(out=out[:, 0:H], in_=res_sb[:, 0:H])
            nc.scalar.activation(e[:, BROWS + H : W], d[:, BROWS + H : W], Exp, scale=EXP_SCALE)
            nc.tensor.matmul(
                psum_r[:], lhsT=e[:, 0:BROWS], rhs=e[:, BROWS + H : W],
                start=False, stop=True,
            )
            nc.vector.tensor_copy(out=res_sb[:, H:RES], in_=psum_r[:])
            nc.scalar.dma_start(out=out[:, H:RES], in_=res_sb[:, H:RES])

    for inst, sem in deferred_waits:
        for wt in inst.ins.sync_info.on_wait:
            if wt.id == sem.num:
                wt.wait_value = 16

    # Hoist the cvbt DMA to the top of the main block, before the framework
    # entry barrier, so it overlaps the per-engine NRT preamble.
    main_blk = nc.m.functions[0].blocks[0]
    insts = main_blk.instructions
    idx = next(i for i, ins in enumerate(insts) if ins.name == cv_dma.ins.name)
    dma_ins = insts.pop(idx)
    insts.insert(1, dma_ins)  # right after the Call
    main_blk.instructions = insts

    # After the tile exit barriers: reset the manual input sem so a
    # re-execution of this loaded NEFF sees it at zero.
    nc.sync.sem_clear(cvbt_sem)

    nc.compile()
    return nc


def _make_inputs(curves: np.ndarray):
    """Per-core input maps."""
    bt = _bernstein_basis()
    xs = (RES * curves[:, :, 0]).astype(np.float32)  # [8,4] = 512*x control pts
    ys = (RES * curves[:, :, 1]).astype(np.float32)

    in_maps = []
    for k in range(N_CORES):
        ysk = ys.T - np.float32(BROWS * k)
        cvbt = np.empty((4, 3 * N_CURVES + 1 + STEPS), dtype=np.float32)
        cvbt[:, 0:N_CURVES] = xs.T
        cvbt[:, N_CURVES] = xs.T[:, N_CURVES - 1] - np.float32(RES // 2)
        cvbt[:, N_CURVES + 1 : 2 * N_CURVES + 1] = ysk
        cvbt[:, 2 * N_CURVES + 1 : 3 * N_CURVES + 1] = -ysk
        cvbt[:, 3 * N_CURVES + 1 :] = bt
        in_maps.append({"cvbt": cvbt})
    return in_maps


def kernel(curves: np.ndarray, trace: bool = False, tmpdir: str | None = None):
    _install_walrus_args_patch()
    _install_ntff_hook()
    from concourse.bass_utils import run_bass_kernel_spmd

    if "nc" not in _CACHE:
        _CACHE["nc"] = build_bass()
    nc = _CACHE["nc"]

    in_maps = _make_inputs(np.asarray(curves, dtype=np.float32))
    kw = {}
    if trace:
        import concourse.bass_utils as bu

        bu.upload_artifacts = lambda d: d  # no bucket in this container
        kw = {"trace": True, "tmpdir": tmpdir}
    res = run_bass_kernel_spmd(nc, in_maps, core_ids=list(range(N_CORES)), **kw)

    full = np.concatenate([res.results[k]["out"] for k in range(N_CORES)], axis=0)
    if trace:
        return full, res
    return full

